# revision 11
# baseline (speedup 1.0000x reference)
"""Trainium2 Bass kernel for nn_CoreBlock (circulant attention + 2-layer FFN).

Contract: kernel(**inputs) takes FULL unsharded inputs (as produced by
setup_inputs) and returns the FULL [16, 1024, 768] f32 output.

Strategy: pure data-parallel over batch — 8 NeuronCores x 2 batches each.
All weights replicated. Per core:
  phase A: LayerNorm(x) -> u, PE-transpose u -> u_dt, v = u_dt.T @ Wv' (per
           token-chunk), results gathered into a resident V tensor in SBUF.
  phase B: per head h: circulant matmul y[h] = C[h] @ v[h] using an 8-tile
           Toeplitz bank T[h,m] (m = (jc-ic) mod 8) precomputed on host;
           residual-added in place into X (X becomes x1 = x + y).
  phase C: 2x [Dense -> LayerNorm -> swish] with PE transposes between
           layers, then log_cosh(z + x1) via Exp/Ln on the scalar engine.

Matmul operands are bf16 (full-rate PE, fp32 PSUM accumulation); stats and
elementwise math are fp32.
"""

import math
import numpy as np
import ml_dtypes

import concourse.bass as bass
import concourse.tile as tile
from concourse import bacc, mybir
from concourse.bass_utils import run_bass_kernel_spmd

BF16 = ml_dtypes.bfloat16

B, N, D = 16, 1024, 768
H, HS, L = 12, 64, 2
EPS = 1e-6
NCORES = 8
BPC = B // NCORES          # batches per core
NJ = N // 128              # token chunks per batch (8)
NT = BPC * NJ              # token chunks per core (16)
DC = D // 128              # feature chunks (6)

F32 = mybir.dt.float32
BF = mybir.dt.bfloat16
Alu = mybir.AluOpType
Act = mybir.ActivationFunctionType

TRACE = False              # test harness sets this for profiling runs
TRACE_KW = {}
DEBUG_DUMPS = False

_cache = {}


def _build(cv_nonzero, bf_nonzero, lnf_uniform):
    """Construct the per-core Bass program. lnf_uniform: list of
    (cs, cb) per FFN layer if lnf scale/bias are uniform, else None entries."""
    nc = bacc.Bacc("TRN2", target_bir_lowering=False, debug=False)

    xs = nc.dram_tensor("xs", (BPC, N, D), F32, kind="ExternalInput").ap()
    wv = nc.dram_tensor("wv", (D, D), BF, kind="ExternalInput").ap()
    wf = nc.dram_tensor("wf", (L, D, D), BF, kind="ExternalInput").ap()
    tb_d = nc.dram_tensor("tbank", (H, 128, NJ * 128), BF, kind="ExternalInput").ap()
    id32 = nc.dram_tensor("id32", (128, 128), F32, kind="ExternalInput").ap()
    idbf = nc.dram_tensor("idbf", (128, 128), BF, kind="ExternalInput").ap()
    cv_d = nc.dram_tensor("cv", (D,), F32, kind="ExternalInput").ap()
    bf_d = nc.dram_tensor("bfb", (L, D), F32, kind="ExternalInput").ap()
    lnfs_d = nc.dram_tensor("lnfs", (L, D), F32, kind="ExternalInput").ap()
    lnfb_d = nc.dram_tensor("lnfb", (L, D), F32, kind="ExternalInput").ap()
    out_d = nc.dram_tensor("out", (BPC, N, D), F32, kind="ExternalOutput").ap()
    dbg = None
    if DEBUG_DUMPS:
        dbg = {
            "dbg_u": nc.dram_tensor("dbg_u", (NT, 128, D), BF, kind="ExternalOutput").ap(),
            "dbg_v": nc.dram_tensor("dbg_v", (128, H, NJ, BPC, HS), BF, kind="ExternalOutput").ap(),
            "dbg_x1": nc.dram_tensor("dbg_x1", (128, BPC, NJ, D), F32, kind="ExternalOutput").ap(),
            "dbg_y1": nc.dram_tensor("dbg_y1", (NT, 128, D), BF, kind="ExternalOutput").ap(),
            "dbg_z1": nc.dram_tensor("dbg_z1", (NT, 128, D), BF, kind="ExternalOutput").ap(),
        }
    with tile.TileContext(nc) as tc:
        _emit(nc, tc, xs, wv, wf, tb_d, id32, idbf, cv_d, bf_d, lnfs_d, lnfb_d,
              out_d, cv_nonzero, bf_nonzero, lnf_uniform, dbg)
    nc.compile()
    return nc


def _emit(nc, tc, xs, wv, wf, tb_d, id32, idbf, cv_d, bf_d, lnfs_d, lnfb_d,
          out_d, cv_nonzero, bf_nonzero, lnf_uniform, dbg=None):
    from contextlib import ExitStack
    ctx = ExitStack()
    with ctx:
        consts = ctx.enter_context(tc.tile_pool(name="consts", bufs=1))
        xpool = ctx.enter_context(tc.tile_pool(name="xpool", bufs=1))
        vpool = ctx.enter_context(tc.tile_pool(name="vpool", bufs=1))
        acts = ctx.enter_context(tc.tile_pool(name="acts", bufs=18))
        tbp = ctx.enter_context(tc.tile_pool(name="tbp", bufs=2))
        upool = ctx.enter_context(tc.tile_pool(name="upool", bufs=3))
        dtp = ctx.enter_context(tc.tile_pool(name="dtp", bufs=3))
        stat = ctx.enter_context(tc.tile_pool(name="stat", bufs=4))
        statp = ctx.enter_context(tc.tile_pool(name="statp", bufs=NT + 2))
        wkp = ctx.enter_context(tc.tile_pool(name="wkp", bufs=3))
        outp = ctx.enter_context(tc.tile_pool(name="outp", bufs=3))
        ps_tr = ctx.enter_context(tc.tile_pool(name="ps_tr", bufs=2, space="PSUM"))
        ps_mm = ctx.enter_context(tc.tile_pool(name="ps_mm", bufs=2, space="PSUM"))

        # ---- constants ----
        wv_s = consts.tile([128, DC, D], BF, tag="wv")
        nc.sync.dma_start(wv_s[:], wv.rearrange("(c p) f -> p c f", p=128))
        wf_s = consts.tile([128, L, DC, D], BF, tag="wf")
        nc.sync.dma_start(wf_s[:], wf.rearrange("l (c p) f -> p l c f", p=128))
        i32 = consts.tile([128, 128], F32, tag="i32")
        nc.sync.dma_start(i32[:], id32)
        ibf = consts.tile([128, 128], BF, tag="ibf")
        nc.sync.dma_start(ibf[:], idbf)
        epst = consts.tile([128, 1], F32, tag="eps")
        nc.vector.memset(epst[:], EPS)
        zerot = consts.tile([128, 1], F32, tag="zero")
        nc.vector.memset(zerot[:], 0.0)
        halft = consts.tile([128, 1], F32, tag="half")
        nc.vector.memset(halft[:], 0.5)
        cvt = None
        if cv_nonzero:
            cvt = consts.tile([128, D], F32, tag="cv")
            nc.sync.dma_start(cvt[:], cv_d.to_broadcast((128, D)))
        bft = [None] * L
        lnfst = [None] * L
        lnfbt = [None] * L
        for l in range(L):
            if bf_nonzero[l]:
                bft[l] = consts.tile([128, D], F32, tag=f"bf{l}")
                nc.sync.dma_start(bft[l][:], bf_d[l].to_broadcast((128, D)))
            if lnf_uniform[l] is None:
                lnfst[l] = consts.tile([128, D], F32, tag=f"lnfs{l}")
                nc.sync.dma_start(lnfst[l][:], lnfs_d[l].to_broadcast((128, D)))
                lnfbt[l] = consts.tile([128, D], F32, tag=f"lnfb{l}")
                nc.sync.dma_start(lnfbt[l][:], lnfb_d[l].to_broadcast((128, D)))

        # ---- resident tensors ----
        X = xpool.tile([128, BPC, NJ, D], F32, tag="X")         # x, then x1
        V = vpool.tile([128, H, NJ, BPC, HS], BF, tag="V")      # per-head values

        # ================= phase A: LN + v-projection =================
        for b in range(BPC):
            for jc in range(NJ):
                xt = X[:, b, jc, :]
                nc.sync.dma_start(xt, xs[b, jc * 128:(jc + 1) * 128, :])
                st = stat.tile([128, 3, 6], F32, tag="bst")
                for g in range(3):
                    nc.vector.bn_stats(st[:, g, :], xt[:, g * 256:(g + 1) * 256])
                mv = stat.tile([128, 2], F32, tag="mv")
                nc.vector.bn_aggr(mv[:], st[:])
                sd = stat.tile([128, 1], F32, tag="sd")
                nc.scalar.activation(sd[:], mv[:, 1:2], Act.Sqrt, bias=epst[:])
                rs = stat.tile([128, 1], F32, tag="rs")
                nc.vector.reciprocal(rs[:], sd[:])
                u = upool.tile([128, D], BF, tag="u")
                nc.vector.tensor_scalar(u[:], xt, mv[:, 0:1], rs[:],
                                        op0=Alu.subtract, op1=Alu.mult)
                if dbg is not None:
                    nc.sync.dma_start(dbg["dbg_u"][b * NJ + jc], u[:])
                ptr = ps_tr.tile([128, D], BF, tag="tr")
                for c in range(DC):
                    nc.tensor.transpose(ptr[:, c * 128:(c + 1) * 128],
                                        u[:, c * 128:(c + 1) * 128], ibf[:])
                udt = dtp.tile([128, D], BF, tag="udt")
                nc.scalar.copy(udt[:], ptr[:])
                pv = ps_mm.tile([128, D], F32, tag="mm")
                for c in range(DC):
                    nc.tensor.matmul(pv[:, 0:512], udt[:, c * 128:(c + 1) * 128],
                                     wv_s[:, c, 0:512],
                                     start=(c == 0), stop=(c == DC - 1))
                    nc.tensor.matmul(pv[:, 512:D], udt[:, c * 128:(c + 1) * 128],
                                     wv_s[:, c, 512:D],
                                     start=(c == 0), stop=(c == DC - 1))
                vdst = V[:, :, jc, b, :]                         # [128, H, HS]
                pv3 = pv[:].rearrange("p (h k) -> p h k", h=H)
                if cv_nonzero:
                    cv3 = cvt[:].rearrange("p (h k) -> p h k", h=H)
                    nc.vector.tensor_tensor(vdst, pv3, cv3, op=Alu.add)
                else:
                    nc.vector.tensor_copy(vdst, pv3)

        # ================= phase B: circulant attention + residual ====
        for h in range(H):
            tb = tbp.tile([128, NJ, 128], BF, tag="tb")
            nc.sync.dma_start(tb[:], tb_d[h].rearrange("p (m f) -> p m f", m=NJ))
            pc = ps_mm.tile([128, NJ, BPC, HS], F32, tag="mm")
            # PSUM `start=True` clears the whole bank, so only the first
            # matmul touching each bank sets it; first writes to the other
            # regions overwrite anyway (has_written starts cleared).
            for m in range(NJ):
                for ic in range(NJ):
                    jc = (ic + m) % NJ
                    nc.tensor.matmul(pc[:, ic, :, :], tb[:, m, :],
                                     V[:, h, jc, :, :],
                                     start=(m == 0 and ic % 4 == 0),
                                     stop=(m == NJ - 1),
                                     skip_group_check=True)
            xap = X[:, :, :, h * HS:(h + 1) * HS]                # [128,BPC,NJ,HS]
            pcr = pc[:].rearrange("p i b k -> p b i k")
            nc.vector.tensor_tensor(xap, xap, pcr, op=Alu.add)

        if dbg is not None:
            nc.sync.dma_start(dbg["dbg_v"][:], V[:])
            nc.sync.dma_start(dbg["dbg_x1"][:], X[:])

        # ================= phase C: FFN x2 + log_cosh =================
        zcur = [None] * NT      # activation tiles per chunk
        mvs = [None] * NT
        for l in range(L):
            fast = lnf_uniform[l] is not None
            for t in range(NT):
                b, jc = divmod(t, NJ)
                src = X[:, b, jc, :] if l == 0 else zcur[t][:]
                ptr = ps_tr.tile([128, D], F32 if l == 0 else BF, tag="tr")
                ident = i32 if l == 0 else ibf
                for c in range(DC):
                    nc.tensor.transpose(ptr[:, c * 128:(c + 1) * 128],
                                        src[:, c * 128:(c + 1) * 128], ident[:])
                zdt = dtp.tile([128, D], BF, tag="zdt")
                nc.scalar.copy(zdt[:], ptr[:])
                pf = ps_mm.tile([128, D], F32, tag="mm")
                for c in range(DC):
                    nc.tensor.matmul(pf[:, 0:512], zdt[:, c * 128:(c + 1) * 128],
                                     wf_s[:, l, c, 0:512],
                                     start=(c == 0), stop=(c == DC - 1))
                    nc.tensor.matmul(pf[:, 512:D], zdt[:, c * 128:(c + 1) * 128],
                                     wf_s[:, l, c, 512:D],
                                     start=(c == 0), stop=(c == DC - 1))
                if bf_nonzero[l]:
                    nc.vector.tensor_tensor(pf[:], pf[:], bft[l][:], op=Alu.add)
                st = stat.tile([128, 3, 6], F32, tag="bst")
                for g in range(3):
                    nc.vector.bn_stats(st[:, g, :], pf[:, g * 256:(g + 1) * 256])
                mv = statp.tile([128, 2], F32, tag=f"mvf{l}")
                nc.vector.bn_aggr(mv[:], st[:])
                mvs[t] = mv
                y = acts.tile([128, D], BF, tag="acts")
                nc.scalar.copy(y[:], pf[:])
                if dbg is not None and l == 0:
                    nc.sync.dma_start(dbg["dbg_y1"][t], y[:])
                zcur[t] = y
            # batched sqrt (one ACT table set), then batched Silu
            sds = [None] * NT
            for t in range(NT):
                sd = statp.tile([128, 1], F32, tag=f"sdf{l}")
                nc.scalar.activation(sd[:], mvs[t][:, 1:2], Act.Sqrt, bias=epst[:])
                sds[t] = sd
            for t in range(NT):
                rs = stat.tile([128, 1], F32, tag=f"rsf{l}")
                nc.vector.reciprocal(rs[:], sds[t][:])
                y = zcur[t]
                if fast:
                    cs, cb = lnf_uniform[l]
                    sc = stat.tile([128, 1], F32, tag=f"scf{l}")
                    if cs != 1.0:
                        nc.vector.tensor_scalar(sc[:], rs[:], float(cs), None,
                                                op0=Alu.mult)
                    else:
                        sc = rs
                    bia = stat.tile([128, 1], F32, tag=f"bif{l}")
                    nc.vector.tensor_scalar(bia[:], mvs[t][:, 0:1], sc[:],
                                            float(-1.0), op0=Alu.mult, op1=Alu.mult)
                    if cb != 0.0:
                        nc.vector.tensor_scalar(bia[:], bia[:], float(cb), None,
                                                op0=Alu.add)
                    nc.scalar.activation(y[:], y[:], Act.Silu,
                                         bias=bia[:], scale=sc[:])
                else:
                    tmp = acts.tile([128, D], BF, tag="acts")
                    nc.vector.tensor_scalar(tmp[:], y[:], mvs[t][:, 0:1], rs[:],
                                            op0=Alu.subtract, op1=Alu.mult)
                    nc.vector.tensor_tensor(tmp[:], tmp[:], lnfst[l][:],
                                            op=Alu.mult)
                    nc.vector.tensor_tensor(tmp[:], tmp[:], lnfbt[l][:],
                                            op=Alu.add)
                    nc.scalar.activation(tmp[:], tmp[:], Act.Silu, bias=zerot[:])
                    zcur[t] = tmp

        if dbg is not None:
            for t in range(NT):
                nc.sync.dma_start(dbg["dbg_z1"][t], zcur[t][:])

        ln2 = math.log(2.0)
        for t in range(NT):
            b, jc = divmod(t, NJ)
            w = wkp.tile([128, D], F32, tag="w")
            nc.vector.tensor_tensor(w[:], zcur[t][:], X[:, b, jc, :], op=Alu.add)
            # |w| = max(w, -w)
            nc.vector.scalar_tensor_tensor(w[:], w[:], -1.0, w[:],
                                           op0=Alu.mult, op1=Alu.max)
            e = wkp.tile([128, D], F32, tag="e")
            nc.scalar.activation(e[:], w[:], Act.Exp, bias=zerot[:], scale=-2.0)
            nc.scalar.activation(e[:], e[:], Act.Ln, bias=halft[:], scale=0.5)
            # e now holds Ln(0.5*exp(-2|w|) + 0.5) = log1p(exp(-2|w|)) - log2
            ot = outp.tile([128, D], F32, tag="ot")
            nc.vector.tensor_tensor(ot[:], w[:], e[:], op=Alu.add)
            nc.sync.dma_start(out_d[b, jc * 128:(jc + 1) * 128, :], ot[:])


def _prep(inputs):
    x = np.asarray(inputs["x"], np.float32)
    ln1_s = np.asarray(inputs["ln1_scale"], np.float32)
    ln1_b = np.asarray(inputs["ln1_bias"], np.float32)
    Wv = np.asarray(inputs["Wv"], np.float32)
    alpha = np.asarray(inputs["alpha"], np.float32)
    Wf = np.asarray(inputs["Wf"], np.float32)
    bfv = np.asarray(inputs["bf"], np.float32)
    lnf_s = np.asarray(inputs["lnf_scale"], np.float32)
    lnf_b = np.asarray(inputs["lnf_bias"], np.float32)

    Wv_flat = Wv.transpose(1, 0, 2).reshape(D, H * HS)
    Wvp = (ln1_s[:, None] * Wv_flat).astype(BF16)
    cv = (ln1_b @ Wv_flat).astype(np.float32)

    ar = alpha[:, (-np.arange(N)) % N]
    ar2 = np.concatenate([ar, ar], axis=1)
    m_ = np.arange(NJ)[:, None, None]
    p_ = np.arange(128)[None, :, None]
    f_ = np.arange(128)[None, None, :]
    T = ar2[:, N + 128 * m_ + p_ - f_]                  # [H, NJ, 128, 128]
    tbank = np.ascontiguousarray(
        T.transpose(0, 2, 1, 3).reshape(H, 128, NJ * 128)).astype(BF16)

    cv_nonzero = bool(np.any(cv))
    bf_nonzero = tuple(bool(np.any(bfv[l])) for l in range(L))
    lnf_uniform = []
    for l in range(L):
        s, bb = lnf_s[l], lnf_b[l]
        if np.all(s == s[0]) and np.all(bb == bb[0]):
            lnf_uniform.append((float(s[0]), float(bb[0])))
        else:
            lnf_uniform.append(None)
    key = (cv_nonzero, bf_nonzero, tuple(lnf_uniform))

    common = {
        "wv": np.ascontiguousarray(Wvp),
        "wf": Wf.astype(BF16),
        "tbank": tbank,
        "id32": np.eye(128, dtype=np.float32),
        "idbf": np.eye(128, dtype=BF16),
        "cv": cv,
        "bfb": bfv,
        "lnfs": lnf_s,
        "lnfb": lnf_b,
    }
    return x, key, common, (cv_nonzero, bf_nonzero, lnf_uniform)


def kernel(**inputs):
    x, key, common, flags = _prep(inputs)
    if key not in _cache:
        _cache[key] = _build(*flags)
    nc = _cache[key]
    in_maps = []
    for i in range(NCORES):
        m = dict(common)
        m["xs"] = np.ascontiguousarray(x[i * BPC:(i + 1) * BPC])
        in_maps.append(m)
    res = run_bass_kernel_spmd(nc, in_maps, core_ids=list(range(NCORES)),
                               trace=TRACE, **TRACE_KW)
    kernel.last_result = res
    out = np.empty((B, N, D), np.float32)
    for i in range(NCORES):
        out[i * BPC:(i + 1) * BPC] = res.results[i]["out"]
    return out
